# revision 1
# baseline (speedup 1.0000x reference)
"""MicroSegHead Trainium2 kernel.

Data-parallel over B*N rows: 8 cores x 512 rows each, params replicated.
Per core, per head h: 3x ([512,2048] @ [2048,2048] + BN + ReLU) then a
final [2048 -> cls_h] projection. Activations live in SBUF transposed
([channel, row]); weights stream from HBM in 2MB pre-swizzled chunks.
Matmuls run as float32r (full-rate PE, ~1e-4 rel err vs fp32).
"""

import os
import sys
import types

import numpy as np

import concourse.bacc as bacc
import concourse.mybir as mybir
import concourse.tile as tile
from concourse.bass_utils import run_bass_kernel_spmd


def _ensure_trace_hook():
    """If BASS_TRACE is set but antenv.axon_hooks is missing (this image),
    install the same ctypes NTFF hook trn_boot.py would; else disable
    tracing so run_bass_kernel_spmd doesn't crash on the import."""
    if os.environ.get("BASS_TRACE", "") in ("", "0"):
        return
    try:
        import antenv.axon_hooks  # noqa: F401
        return
    except ImportError:
        pass
    try:
        import antenv
        sys.path.insert(0, "/root/.axon_site")
        from trn_agent_boot.trn_boot import _ntff_profile_via_ctypes
        hook = _ntff_profile_via_ctypes("/opt/axon/libaxon_pjrt.so")
        mod = types.ModuleType("antenv.axon_hooks")
        mod.get_axon_ntff_profile_hook = lambda: hook
        mod.set_axon_ntff_profile_hook = lambda h: None
        sys.modules["antenv.axon_hooks"] = mod
        antenv.axon_hooks = mod
    except Exception:
        os.environ["BASS_NEVER_TRACE"] = "1"

B, N, C = 16, 256, 2048
CLASSES = (16, 5, 5)
H = 3
EPS = 1e-5
NCORES = 8
M = (B * N) // NCORES          # 512 rows per core
CT = C // 128                  # 16 contraction tiles
DT = C // 128                  # 16 output-channel tiles
QD = 2                         # d-tiles per weight DMA chunk
NQ = DT // QD                  # 8 chunks per stage
NSTAGES = H * 3

F32 = mybir.dt.float32
F32R = mybir.dt.float32r

LAST = {"exec_time_ns": None}

_PROG = None
_WCACHE = {}


def _build_program():
    nc = bacc.Bacc("TRN2", target_bir_lowering=False)

    x_d = nc.dram_tensor("x", [128, CT, M], F32R, kind="ExternalInput")
    w_ds = [
        nc.dram_tensor(f"w{s}", [NQ, 128, QD, CT * 128], F32R, kind="ExternalInput")
        for s in range(NSTAGES)
    ]
    sc_d = nc.dram_tensor("sc", [128, NSTAGES, DT], F32, kind="ExternalInput")
    sh_d = nc.dram_tensor("sh", [128, NSTAGES, DT], F32, kind="ExternalInput")
    wf_ds = [
        nc.dram_tensor(f"wf{h}", [128, CT, CLASSES[h]], F32R, kind="ExternalInput")
        for h in range(H)
    ]
    bf_ds = [
        nc.dram_tensor(f"bf{h}", [CLASSES[h], 1], F32, kind="ExternalInput")
        for h in range(H)
    ]
    out_ds = [
        nc.dram_tensor(f"out{h}", [CLASSES[h], M], F32, kind="ExternalOutput")
        for h in range(H)
    ]

    with tile.TileContext(nc) as tc:
        with (
            tc.tile_pool(name="xpool", bufs=1) as xpool,
            tc.tile_pool(name="ypool", bufs=1) as ypool,
            tc.tile_pool(name="wpool", bufs=5) as wpool,
            tc.tile_pool(name="cpool", bufs=1) as cpool,
            tc.tile_pool(name="opool", bufs=2) as opool,
            tc.tile_pool(name="psum", bufs=6, space="PSUM") as ppool,
            tc.tile_pool(name="psumf", bufs=2, space="PSUM") as fpool,
        ):
            # Startup: the first matmul chain needs x c-tiles + the first
            # weight d-block. Issue x from the Scalar engine so its DMA
            # issues overlap Sync's, and split the first weight chunk so
            # the d0 block lands first.
            x_sb = xpool.tile([128, CT, M], F32R)
            w0_sb = wpool.tile([128, QD, CT * 128], F32R, tag="w")
            nc.sync.dma_start(w0_sb[:, 0, :], w_ds[0][0][:, 0, :])
            XP = 4
            for part in range(XP):
                cs = part * (CT // XP)
                ce = cs + CT // XP
                nc.scalar.dma_start(x_sb[:, cs:ce, :], x_d[:, cs:ce, :])
            nc.sync.dma_start(w0_sb[:, 1, :], w_ds[0][0][:, 1, :])
            sc_sb = cpool.tile([128, NSTAGES, DT], F32)
            sh_sb = cpool.tile([128, NSTAGES, DT], F32)
            nc.sync.dma_start(sc_sb[:], sc_d[:])
            nc.sync.dma_start(sh_sb[:], sh_d[:])
            wf_sbs = [None] * H
            bf_sbs = [None] * H

            for h in range(H):
                src = x_sb
                for layer in range(3):
                    s = h * 3 + layer
                    dst = ypool.tile([128, DT, M], F32R, tag="ya" if layer % 2 == 0 else "yb")
                    for q in range(NQ):
                        if s == 0 and q == 0:
                            # Split each d-chain into c halves so the PE
                            # starts once half of x has landed.
                            psums = [ppool.tile([128, M], F32, tag="ps",
                                                name=f"ps0_{i}")
                                     for i in range(QD)]
                            for half in range(2):
                                for dd in range(QD):
                                    for c in range(half * 8, half * 8 + 8):
                                        nc.tensor.matmul(
                                            psums[dd][:],
                                            lhsT=w0_sb[:, dd, c * 128:(c + 1) * 128],
                                            rhs=src[:, c, :],
                                            start=(c == 0),
                                            stop=(c == CT - 1),
                                        )
                            for dd in range(QD):
                                nc.scalar.activation(
                                    dst[:, dd, :], psums[dd][:],
                                    mybir.ActivationFunctionType.Relu,
                                    bias=sh_sb[:, s, dd:dd + 1],
                                    scale=sc_sb[:, s, dd:dd + 1],
                                )
                            continue
                        w_sb = wpool.tile([128, QD, CT * 128], F32R, tag="w")
                        nc.sync.dma_start(w_sb[:], w_ds[s][q])
                        for dd in range(QD):
                            d = q * QD + dd
                            psum = ppool.tile([128, M], F32, tag="ps")
                            for c in range(CT):
                                nc.tensor.matmul(
                                    psum[:],
                                    lhsT=w_sb[:, dd, c * 128:(c + 1) * 128],
                                    rhs=src[:, c, :],
                                    start=(c == 0),
                                    stop=(c == CT - 1),
                                )
                            nc.scalar.activation(
                                dst[:, d, :], psum[:],
                                mybir.ActivationFunctionType.Relu,
                                bias=sh_sb[:, s, d:d + 1],
                                scale=sc_sb[:, s, d:d + 1],
                            )
                    src = dst

                cls = CLASSES[h]
                wf_sb = cpool.tile([128, CT, cls], F32R, tag=f"wf{h}")
                nc.sync.dma_start(wf_sb[:], wf_ds[h][:])
                wf_sbs[h] = wf_sb
                bf_sb = cpool.tile([cls, 1], F32, tag=f"bf{h}")
                nc.sync.dma_start(bf_sb[:], bf_ds[h][:])
                bf_sbs[h] = bf_sb
                psf = fpool.tile([cls, M], F32, tag="pf")
                for c in range(CT):
                    nc.tensor.matmul(
                        psf[:],
                        lhsT=wf_sbs[h][:, c, :],
                        rhs=src[:, c, :],
                        start=(c == 0),
                        stop=(c == CT - 1),
                    )
                o_sb = opool.tile([cls, M], F32, tag="of")
                nc.vector.tensor_tensor(
                    o_sb[:], psf[:],
                    bf_sbs[h][:, :].to_broadcast((cls, M)),
                    mybir.AluOpType.add,
                )
                nc.sync.dma_start(out_ds[h][:], o_sb[:])

    nc.compile()
    return nc


def _get_prog():
    global _PROG
    if _PROG is None:
        _PROG = _build_program()
    return _PROG


def _swizzle_w(W_h):
    """[d, c] (2048x2048) -> [NQ, 128, QD, CT*128] with
    out[q, p, dd, ct*128 + j] = W_h[(q*QD+dd)*128 + j, ct*128 + p]."""
    W4 = W_h.reshape(DT, 128, CT, 128)          # [dt, dj, ct, cj]
    A = W4.transpose(0, 3, 2, 1)                # [dt, p, ct, j]
    Bv = A.reshape(NQ, QD, 128, CT, 128)        # [q, dd, p, ct, j]
    return np.ascontiguousarray(Bv.transpose(0, 2, 1, 3, 4)).reshape(
        NQ, 128, QD, CT * 128
    )


def kernel(features, W1, g1, b1, m1, v1, W2, g2, b2, m2, v2, W3, g3, b3, m3, v3,
           Wf0, bf0, Wf1, bf1, Wf2, bf2):
    features = np.asarray(features, dtype=np.float32)
    Ws = [np.asarray(W, dtype=np.float32) for W in (W1, W2, W3)]
    gs = [np.asarray(a, dtype=np.float32) for a in (g1, g2, g3)]
    bs = [np.asarray(a, dtype=np.float32) for a in (b1, b2, b3)]
    ms = [np.asarray(a, dtype=np.float32) for a in (m1, m2, m3)]
    vs = [np.asarray(a, dtype=np.float32) for a in (v1, v2, v3)]
    Wfs = [np.asarray(W, dtype=np.float32) for W in (Wf0, Wf1, Wf2)]
    bfs = [np.asarray(a, dtype=np.float32) for a in (bf0, bf1, bf2)]

    nc = _get_prog()

    ck = (Ws[0].ravel()[:16].tobytes(), float(Ws[1][1, 7, 7]),
          float(Ws[2][-1, -1, -1]), float(Wfs[0][0, 0]))
    if _WCACHE.get("key") == ck:
        common = _WCACHE["common"]
    else:
        common = {}
        sc_all = np.empty((128, NSTAGES, DT), np.float32)
        sh_all = np.empty((128, NSTAGES, DT), np.float32)
        for h in range(H):
            for layer in range(3):
                s = h * 3 + layer
                common[f"w{s}"] = _swizzle_w(Ws[layer][h])
                scale = gs[layer][h] / np.sqrt(vs[layer][h] + EPS)
                shift = bs[layer][h] - ms[layer][h] * scale
                sc_all[:, s, :] = scale.reshape(DT, 128).T
                sh_all[:, s, :] = shift.reshape(DT, 128).T
        common["sc"] = sc_all
        common["sh"] = sh_all
        for h in range(H):
            cls = CLASSES[h]
            common[f"wf{h}"] = np.ascontiguousarray(
                Wfs[h].reshape(cls, CT, 128).transpose(2, 1, 0)
            )
            common[f"bf{h}"] = bfs[h].reshape(cls, 1)
        _WCACHE["key"] = ck
        _WCACHE["common"] = common

    x_flat = features.reshape(B * N, C)
    in_maps = []
    for core in range(NCORES):
        shard = x_flat[core * M:(core + 1) * M]
        x_sw = np.ascontiguousarray(shard.reshape(M, CT, 128).transpose(2, 1, 0))
        in_maps.append({"x": x_sw, **common})

    _ensure_trace_hook()
    res = run_bass_kernel_spmd(nc, in_maps, core_ids=list(range(NCORES)))
    LAST["exec_time_ns"] = res.exec_time_ns
    LAST["results"] = res

    blocks = []
    for core in range(NCORES):
        r = res.results[core]
        blocks.append(
            np.concatenate([r[f"out{h}"].T for h in range(H)], axis=1)
        )
    out = np.concatenate(blocks, axis=0)       # [B*N, sum(classes)]
    return out.reshape(B, N, sum(CLASSES))

